# revision 1
# baseline (speedup 1.0000x reference)
"""ConvLocalAttention2d (7x7 window, 4 heads, d_head=16) on 8 trn2 NeuronCores.

Sharding: one (batch, head) pair per core  (B=2 x heads=4 = 8 cores), zero
cross-core communication.

Per-core algorithm (d=16, H=W=96, K=7x7=49):
  - Queries are processed in 8x16 = 128-pixel tiles (12 x 6 = 72 tiles).
  - For each tile, the key/value halo is 14x22 = 308 pixels, split into 3
    row-chunks of <=128 keys (5*22, 5*22, 4*22).
  - C1 (scores, transposed): for each chunk, PE matmul
        scoresT[keys, 128q] = k_aug_chunk.T @ q_aug_tile
    contracting over 65 "channels": 16 data channels + 1 image-boundary
    indicator + 16 y-position one-hot (mod 16) + 32 x-position one-hot
    (mod 32).  The position channels add 0 for in-window pairs and -1e9
    for out-of-window pairs, so the 7x7 window mask and the image-boundary
    mask are folded into the matmul itself (exact: the mod-16/mod-32
    aliases never collide within a tile's +-10 / +-18 offset range).
  - exp: one ACT pass per 2 tiles, PSUM->SBUF, scale=1/sqrt(16), out bf16.
  - C2: PE matmul out_u[128q, 17] += expT_chunk.T @ vt_chunk, where vt has
    17 columns: 16 v channels + a column of ones (inside the image) whose
    output is the softmax denominator Z.
  - DVE: rz = 1/Z, out = out_u[:, :16] * rz (per-partition scalar), DMA out.
"""

import functools
import numpy as np
import ml_dtypes

B = 2
HEADS = 4
DH = 16
H = W = 96
PAD = 3
PH = PW = H + 2 * PAD  # 102
TY, TX = 8, 16           # query tile shape
GY, GX = H // TY, W // TX  # 12 x 6 tile grid
HY, HX = TY + 6, TX + 6    # halo 14 x 22
NCH = 1 + 16 + 32          # img + y-onehot(16) + x-onehot(32)
D = DH + NCH               # 65 contraction channels
NEG = -1.0e9
CHUNKS = [(0, 5), (5, 5), (10, 4)]  # (row0, nrows) of halo row chunks

BF16 = ml_dtypes.bfloat16

Y_OK = {d % 16 for d in range(-3, 4)}
X_OK = {d % 32 for d in range(-3, 4)}


@functools.lru_cache(maxsize=1)
def _mask_channels():
    """Constant (core-independent) aug channels.

    q side: [NCH, H, W]  (ones, y-onehot, x-onehot)
    k side: [NCH, PH, PW] (img indicator, y-g-maps, x-g-maps), value 0 where
    the channel does not veto the pair, NEG where it does.
    """
    qm = np.zeros((NCH, H, W), np.float32)
    qm[0] = 1.0
    yy = np.arange(H)
    xx = np.arange(W)
    for r in range(16):
        qm[1 + r, yy % 16 == r, :] = 1.0
    for r in range(32):
        qm[17 + r, :, xx % 32 == r] = 1.0

    km = np.zeros((NCH, PH, PW), np.float32)
    km[0] = NEG
    km[0, PAD:PAD + H, PAD:PAD + W] = 0.0
    py = np.arange(PH)
    px = np.arange(PW)
    for r in range(16):
        bad = np.array([((y - PAD - r) % 16) not in Y_OK for y in py])
        km[1 + r, bad, :] = NEG
    for r in range(32):
        bad = np.array([((x - PAD - r) % 32) not in X_OK for x in px])
        km[17 + r, :, bad] = NEG
    return qm, km


def _host_prep(q, k, v):
    """Full [2,64,96,96] f32 inputs -> list of 8 per-core input dicts."""
    qm, km = _mask_channels()
    in_maps = []
    for core in range(8):
        b, h = divmod(core, HEADS)
        qs = q[b, DH * h:DH * h + DH]          # [16, 96, 96]
        ks = k[b, DH * h:DH * h + DH]
        vs = v[b, DH * h:DH * h + DH]

        q_aug = np.empty((D, H, W), np.float32)
        q_aug[:DH] = qs
        q_aug[DH:] = qm
        # [D, GY, TY, GX, TX] -> [D, GY*GX, TY*TX]
        q_tiled = np.ascontiguousarray(
            q_aug.reshape(D, GY, TY, GX, TX).transpose(0, 1, 3, 2, 4)
            .reshape(D, GY * GX, TY * TX))

        k_aug = np.empty((D, PH, PW), np.float32)
        k_aug[:DH] = 0.0
        k_aug[:DH, PAD:PAD + H, PAD:PAD + W] = ks
        k_aug[DH:] = km
        # per-tile halo, flattened: [D, 72, 308]
        k_tiled = np.empty((D, GY * GX, HY * HX), np.float32)
        for ty in range(GY):
            for tx in range(GX):
                k_tiled[:, ty * GX + tx] = k_aug[
                    :, TY * ty:TY * ty + HY,
                    TX * tx:TX * tx + HX].reshape(D, HY * HX)

        vt = np.zeros((PH, PW, DH + 1), np.float32)
        vt[PAD:PAD + H, PAD:PAD + W, :DH] = np.transpose(vs, (1, 2, 0))
        vt[PAD:PAD + H, PAD:PAD + W, DH] = 1.0
        # key-within-chunk major so the whole tensor loads in one DMA and
        # stays SBUF-resident: [110, 72, 3, 17]
        vt_res = np.zeros((110, GY * GX, 3, DH + 1), np.float32)
        for ty in range(GY):
            for tx in range(GX):
                halo = vt[TY * ty:TY * ty + HY,
                          TX * tx:TX * tx + HX, :].reshape(HY * HX, DH + 1)
                for c, (r0, nr) in enumerate(CHUNKS):
                    nk = nr * HX
                    vt_res[:nk, ty * GX + tx, c, :] = halo[
                        HX * r0:HX * r0 + nk]

        in_maps.append({
            "q_tiled": q_tiled.astype(BF16),
            "k_tiled": k_tiled.astype(BF16),
            "vt_res": vt_res.astype(BF16),
        })
    return in_maps


@functools.lru_cache(maxsize=1)
def _build_program():
    from contextlib import ExitStack
    import concourse.bass as bass
    import concourse.tile as tile
    from concourse import bacc, mybir

    f32 = mybir.dt.float32
    bf16 = mybir.dt.bfloat16

    nc = bacc.Bacc("TRN2", target_bir_lowering=False, debug=False,
                   num_devices=8)
    q_d = nc.dram_tensor("q_tiled", [D, GY * GX, TY * TX], bf16,
                         kind="ExternalInput").ap()
    k_d = nc.dram_tensor("k_tiled", [D, GY * GX, HY * HX], bf16,
                         kind="ExternalInput").ap()
    vt_d = nc.dram_tensor("vt_res", [110, GY * GX, 3, DH + 1], bf16,
                          kind="ExternalInput").ap()
    # flat [query-in-tile, tile, d] order; host un-permutes
    out_d = nc.dram_tensor("out", [TY * TX, GY * GX, DH], f32,
                           kind="ExternalOutput").ap()

    with tile.TileContext(nc) as tc:
        with ExitStack() as ctx:
            cpool = ctx.enter_context(tc.tile_pool(name="const", bufs=1))
            spool = ctx.enter_context(tc.tile_pool(name="sb", bufs=2))
            vpool = ctx.enter_context(tc.tile_pool(name="vt", bufs=6))
            opool = ctx.enter_context(tc.tile_pool(name="osb", bufs=3))
            zpool = ctx.enter_context(tc.tile_pool(name="rz", bufs=3))
            pp_s = ctx.enter_context(
                tc.tile_pool(name="ps_scores", bufs=2, space="PSUM"))
            pp_o = ctx.enter_context(
                tc.tile_pool(name="ps_out", bufs=2, space="PSUM"))

            NT = GY * GX
            q_sb = cpool.tile([D, NT, TY * TX], bf16)
            k_sb = cpool.tile([D, NT, HY * HX], bf16)
            vt_sb = cpool.tile([110, NT, 3, DH + 1], bf16)
            out_all = cpool.tile([TY * TX, NT, DH], f32)
            # sliced loads so early tiles unblock before the full load lands
            GRP = 12
            for g in range(NT // GRP):
                s = slice(GRP * g, GRP * (g + 1))
                nc.sync.dma_start(q_sb[:, s, :], q_d[:, s, :])
                nc.sync.dma_start(k_sb[:, s, :], k_d[:, s, :])
                nc.sync.dma_start(vt_sb[:, s, :, :], vt_d[:, s, :, :])

            for pair in range(NT // 2):
                scores = pp_s.tile([128, 6, 128], f32, tag="scores")
                for half in range(2):
                    t = 2 * pair + half
                    for c, (r0, nr) in enumerate(CHUNKS):
                        nk = nr * HX
                        nc.tensor.matmul(
                            scores[:nk, 3 * half + c, :],
                            lhsT=k_sb[:, t, HX * r0:HX * r0 + nk],
                            rhs=q_sb[:, t, :],
                            start=True, stop=True)
                expT = spool.tile([128, 6, 128], bf16, tag="expT")
                nc.scalar.activation(expT[:], scores[:],
                                     mybir.ActivationFunctionType.Exp,
                                     scale=0.25)
                for half in range(2):
                    t = 2 * pair + half
                    out_u = pp_o.tile([128, DH + 1], f32, tag="outu")
                    for c, (r0, nr) in enumerate(CHUNKS):
                        nk = nr * HX
                        nc.tensor.matmul(
                            out_u[:],
                            lhsT=expT[:nk, 3 * half + c, :],
                            rhs=vt_sb[:nk, t, c, :],
                            start=(c == 0), stop=(c == 2))
                    rz = zpool.tile([128, 1], f32, tag="rz")
                    nc.vector.reciprocal(rz[:], out_u[:, DH:DH + 1])
                    nc.vector.tensor_scalar_mul(out_all[:, t, :],
                                                out_u[:, :DH], rz[:])
            for g in range(NT // GRP):
                s = slice(GRP * g, GRP * (g + 1))
                nc.scalar.dma_start(out_d[:, s, :], out_all[:, s, :])
    nc.compile()
    return nc


def kernel(q, k, v):
    from concourse.bass_utils import run_bass_kernel_spmd

    nc = _build_program()
    in_maps = _host_prep(np.asarray(q, np.float32), np.asarray(k, np.float32),
                         np.asarray(v, np.float32))
    res = run_bass_kernel_spmd(nc, in_maps, list(range(8)))

    out = np.empty((B, HEADS, DH, H, W), np.float32)
    for core in range(8):
        b, h = divmod(core, HEADS)
        # [TY*TX, NT, DH] -> [qy,qx,ty,tx,d] -> [ty,qy,tx,qx,d] -> [H,W,DH]
        o = res.results[core]["out"].reshape(TY, TX, GY, GX, DH)
        o = o.transpose(2, 0, 3, 1, 4).reshape(H, W, DH)
        out[b, h] = np.transpose(o, (2, 0, 1))
    return out.reshape(B, HEADS * DH, H, W)



# revision 2
# speedup vs baseline: 1.1824x; 1.1824x over previous
"""ConvLocalAttention2d (7x7 window, 4 heads, d_head=16) on 8 trn2 NeuronCores.

Sharding: one (batch, head) pair per core  (B=2 x heads=4 = 8 cores), zero
cross-core communication.

Per-core algorithm (d=16, H=W=96, K=7x7=49):
  - Queries are processed in 8x12 = 96-pixel tiles (12 x 8 = 96 tiles).
  - For each tile the key/value halo is 14x18 = 252 pixels, split into TWO
    128-key chunks (keys 0:128 and 128:252 + 4 zero pad).
  - C1 (scores, transposed): per chunk one PE matmul
        scoresT[128keys, 96q] = k_chunk.T @ q_tile
    contracting over 48 channels: 16 data + 14 y-onehot (mod 14) + 18
    x-onehot (mod 18).  The position channels add 0 for in-window pairs and
    -1e9 for out-of-window pairs (exact: mod-14/mod-18 never alias within
    the tile's dy in [-10,10] / dx in [-14,14] ranges).  No image-boundary
    channel is needed: out-of-image keys have k=0 (score 0, exp 1) but their
    v/ones columns are 0, so they contribute nothing to out or Z.
    Each chunk weight load is exactly 128 columns -> FWL fast weight load.
  - exp: one ACT per 6 tiles (12 chunk-slots, [128,12,96] strided), PSUM ->
    SBUF bf16, scale=1/sqrt(16)=0.25.
  - C2: out_u[96q, 17] += expT_chunk.T @ vt_chunk; vt has 16 v channels + a
    ones(inside-image) column whose output is the softmax denominator Z.
  - epilogue, batched per 6 tiles: rz = 1/Z (one DVE reciprocal [96,6]),
    out = out_u[:, :16] * rz (one DVE tensor_tensor with stride-0 broadcast).
  - Program order is software-pipelined: C1 of group g+1 is emitted before
    C2 of group g so the in-order PE queue never stalls behind the ACT.
"""

import functools
import numpy as np
import ml_dtypes

B = 2
HEADS = 4
DH = 16
H = W = 96
PAD = 3
TY, TX = 8, 12             # query tile shape (96 queries)
GY, GX = H // TY, W // TX  # 12 x 8 = 96 tiles
NT = GY * GX
Q = TY * TX                # 96
HY, HX = TY + 6, TX + 6    # halo 14 x 18 = 252 keys
HALO = HY * HX
KPC = 128                  # keys per chunk (2 chunks, zero-padded to 256)
MY, MX = 14, 18            # one-hot moduli (exact, see header)
NCH = MY + MX              # 32 mask channels
D = DH + NCH               # 48 contraction channels
NEG = -1.0e9
GRP = 6                    # tiles per pipeline group
NG = NT // GRP             # 16 groups
LD = 12                    # tiles per input-DMA slice

BF16 = ml_dtypes.bfloat16

_Y_OK = {d % MY for d in range(-3, 4)}
_X_OK = {d % MX for d in range(-3, 4)}


@functools.lru_cache(maxsize=1)
def _mask_channels():
    """Constant aug channels.

    q side: [NCH, H, W] one-hots (y mod MY, x mod MX).
    k side: [NCH, PH, PW] veto values: 0 where the channel does not veto the
    pair, NEG where it does.
    """
    PH = PW = H + 2 * PAD
    qm = np.zeros((NCH, H, W), np.float32)
    yy = np.arange(H)
    xx = np.arange(W)
    for r in range(MY):
        qm[r, yy % MY == r, :] = 1.0
    for r in range(MX):
        qm[MY + r, :, xx % MX == r] = 1.0

    km = np.zeros((NCH, PH, PW), np.float32)
    py = np.arange(PH) - PAD   # absolute y of padded row
    px = np.arange(PW) - PAD
    for r in range(MY):
        bad = np.array([((y - r) % MY) not in _Y_OK for y in py])
        km[r, bad, :] = NEG
    for r in range(MX):
        bad = np.array([((x - r) % MX) not in _X_OK for x in px])
        km[MY + r, :, bad] = NEG
    return qm, km


def _host_prep(q, k, v):
    """Full [2,64,96,96] f32 inputs -> list of 8 per-core input dicts."""
    qm, km = _mask_channels()
    PH = PW = H + 2 * PAD
    in_maps = []
    for core in range(8):
        b, h = divmod(core, HEADS)
        qs = q[b, DH * h:DH * h + DH]          # [16, 96, 96]
        ks = k[b, DH * h:DH * h + DH]
        vs = v[b, DH * h:DH * h + DH]

        q_aug = np.empty((D, H, W), np.float32)
        q_aug[:DH] = qs
        q_aug[DH:] = qm
        # [D, GY, TY, GX, TX] -> [D, NT, Q]
        q_tiled = np.ascontiguousarray(
            q_aug.reshape(D, GY, TY, GX, TX).transpose(0, 1, 3, 2, 4)
            .reshape(D, NT, Q))

        k_aug = np.empty((D, PH, PW), np.float32)
        k_aug[:DH] = 0.0
        k_aug[:DH, PAD:PAD + H, PAD:PAD + W] = ks
        k_aug[DH:] = km
        # per-tile halo, flattened + padded to 256: [D, NT, 256]
        k_tiled = np.zeros((D, NT, 2 * KPC), np.float32)
        for ty in range(GY):
            for tx in range(GX):
                k_tiled[:, ty * GX + tx, :HALO] = k_aug[
                    :, TY * ty:TY * ty + HY,
                    TX * tx:TX * tx + HX].reshape(D, HALO)

        vt = np.zeros((PH, PW, DH + 1), np.float32)
        vt[PAD:PAD + H, PAD:PAD + W, :DH] = np.transpose(vs, (1, 2, 0))
        vt[PAD:PAD + H, PAD:PAD + W, DH] = 1.0
        # [128, NT, 2, 17], chunk c holds halo keys [128c : 128c+128)
        vt_res = np.zeros((KPC, NT, 2, DH + 1), np.float32)
        for ty in range(GY):
            for tx in range(GX):
                halo = vt[TY * ty:TY * ty + HY,
                          TX * tx:TX * tx + HX, :].reshape(HALO, DH + 1)
                vt_res[:, ty * GX + tx, 0] = halo[:KPC]
                vt_res[:HALO - KPC, ty * GX + tx, 1] = halo[KPC:]

        in_maps.append({
            "q_tiled": q_tiled.astype(BF16),
            "k_tiled": k_tiled.astype(BF16),
            "vt_res": vt_res.astype(BF16),
        })
    return in_maps


@functools.lru_cache(maxsize=1)
def _build_program():
    from contextlib import ExitStack
    import concourse.bass as bass
    import concourse.tile as tile
    from concourse import bacc, mybir

    f32 = mybir.dt.float32
    bf16 = mybir.dt.bfloat16

    nc = bacc.Bacc("TRN2", target_bir_lowering=False, debug=False,
                   num_devices=8)
    q_d = nc.dram_tensor("q_tiled", [D, NT, Q], bf16,
                         kind="ExternalInput").ap()
    k_d = nc.dram_tensor("k_tiled", [D, NT, 2 * KPC], bf16,
                         kind="ExternalInput").ap()
    vt_d = nc.dram_tensor("vt_res", [KPC, NT, 2, DH + 1], bf16,
                          kind="ExternalInput").ap()
    # flat [query-in-tile, tile, d] order; host un-permutes
    out_d = nc.dram_tensor("out", [Q, NT, DH], f32,
                           kind="ExternalOutput").ap()

    with tile.TileContext(nc) as tc:
        with ExitStack() as ctx:
            cpool = ctx.enter_context(tc.tile_pool(name="const", bufs=1))
            spool = ctx.enter_context(tc.tile_pool(name="sb", bufs=2))
            zpool = ctx.enter_context(tc.tile_pool(name="rz", bufs=2))
            pp_s = ctx.enter_context(
                tc.tile_pool(name="ps_scores", bufs=2, space="PSUM"))
            pp_o = ctx.enter_context(
                tc.tile_pool(name="ps_out", bufs=2, space="PSUM"))

            q_sb = cpool.tile([D, NT, Q], bf16)
            k_sb = cpool.tile([D, NT, 2 * KPC], bf16)
            vt_sb = cpool.tile([KPC, NT, 2, DH + 1], bf16)
            out_all = cpool.tile([Q, NT, DH], f32)
            # sliced loads so early groups unblock before the full load lands
            for g in range(NT // LD):
                s = slice(LD * g, LD * (g + 1))
                nc.sync.dma_start(q_sb[:, s, :], q_d[:, s, :])
                nc.sync.dma_start(k_sb[:, s, :], k_d[:, s, :])
                nc.sync.dma_start(vt_sb[:, s, :, :], vt_d[:, s, :, :])

            def c1(g):
                scores = pp_s.tile([128, 2 * GRP, 128], f32, tag="scores")
                for i in range(GRP):
                    t = GRP * g + i
                    for c in range(2):
                        nc.tensor.matmul(
                            scores[:, 2 * i + c, :Q],
                            lhsT=k_sb[:, t, KPC * c:KPC * (c + 1)],
                            rhs=q_sb[:, t, :],
                            start=True, stop=True)
                return scores

            scores_cur = c1(0)
            for g in range(NG):
                expT = spool.tile([128, 2 * GRP, 128], bf16, tag="expT")
                nc.scalar.activation(expT[:, :, :Q], scores_cur[:, :, :Q],
                                     mybir.ActivationFunctionType.Exp,
                                     scale=0.25)
                if g + 1 < NG:
                    scores_cur = c1(g + 1)
                outu = pp_o.tile([128, GRP, 32], f32, tag="outu")
                for i in range(GRP):
                    t = GRP * g + i
                    for c in range(2):
                        nc.tensor.matmul(
                            outu[:, i, :DH + 1],
                            lhsT=expT[:, 2 * i + c, :],
                            rhs=vt_sb[:, t, c, :],
                            start=(c == 0), stop=(c == 1))
                rz = zpool.tile([Q, GRP, 1], f32, tag="rz")
                nc.vector.reciprocal(rz[:], outu[:Q, :, DH:DH + 1])
                u_b, rz_b = bass.broadcast_tensor_aps(
                    outu[:Q, :, :DH], rz[:, :, :])
                nc.vector.tensor_tensor(out_all[:, GRP * g:GRP * (g + 1), :],
                                        u_b, rz_b, mybir.AluOpType.mult)
            for g in range(NT // LD):
                s = slice(LD * g, LD * (g + 1))
                nc.gpsimd.dma_start(out_d[:, s, :], out_all[:, s, :])
    nc.compile()
    return nc


def kernel(q, k, v):
    from concourse.bass_utils import run_bass_kernel_spmd

    nc = _build_program()
    in_maps = _host_prep(np.asarray(q, np.float32), np.asarray(k, np.float32),
                         np.asarray(v, np.float32))
    res = run_bass_kernel_spmd(nc, in_maps, list(range(8)))

    out = np.empty((B, HEADS, DH, H, W), np.float32)
    for core in range(8):
        b, h = divmod(core, HEADS)
        # [Q, NT, DH] -> [qy,qx,ty,tx,d] -> [d,ty,qy,tx,qx] -> [DH,H,W]
        o = res.results[core]["out"].reshape(TY, TX, GY, GX, DH)
        out[b, h] = o.transpose(4, 2, 0, 3, 1).reshape(DH, H, W)
    return out.reshape(B, HEADS * DH, H, W)


# revision 5
# speedup vs baseline: 1.2299x; 1.0401x over previous
"""ConvLocalAttention2d (7x7 window, 4 heads, d_head=16) on 8 trn2 NeuronCores.

Sharding: one (batch, head) pair per core  (B=2 x heads=4 = 8 cores), zero
cross-core communication.

Per-core algorithm (d=16, H=W=96, K=7x7=49):
  - Queries are processed in 8x12 = 96-pixel tiles (12 x 8 = 96 tiles).
  - For each tile the key/value halo is 14x18 = 252 pixels, split into TWO
    128-key chunks (keys 0:128 and 128:252 + 4 zero pad).
  - C1 (scores, transposed): per chunk one PE matmul
        scoresT[128keys, 96q] = k_chunk.T @ q_tile
    contracting over 48 channels: 16 data + 14 y-onehot (mod 14) + 18
    x-onehot (mod 18).  The position channels add 0 for in-window pairs and
    -1e9 for out-of-window pairs (exact: mod-14/mod-18 never alias within
    the tile's dy in [-10,10] / dx in [-14,14] ranges).  No image-boundary
    channel is needed: out-of-image keys have k=0 (score 0, exp 1) but their
    v/ones columns are 0, so they contribute nothing to out or Z.
    Each chunk weight load is exactly 128 columns -> FWL fast weight load.
  - exp: one ACT per 6 tiles (12 chunk-slots, [128,12,96] strided), PSUM ->
    SBUF bf16, scale=1/sqrt(16)=0.25.
  - C2: out_u[96q, 17] += expT_chunk.T @ vt_chunk; vt has 16 v channels + a
    ones(inside-image) column whose output is the softmax denominator Z.
  - epilogue, batched per 6 tiles: rz = 1/Z (one DVE reciprocal [96,6]),
    out = out_u[:, :16] * rz (one DVE tensor_tensor with stride-0 broadcast).
  - Program order is software-pipelined: C1 of group g+1 is emitted before
    C2 of group g so the in-order PE queue never stalls behind the ACT.
"""

import functools
import numpy as np
import ml_dtypes

B = 2
HEADS = 4
DH = 16
H = W = 96
PAD = 3
TY, TX = 8, 12             # query tile shape (96 queries)
GY, GX = H // TY, W // TX  # 12 x 8 = 96 tiles
NT = GY * GX
Q = TY * TX                # 96
HY, HX = TY + 6, TX + 6    # halo 14 x 18 = 252 keys
HALO = HY * HX
KPC = 128                  # keys per chunk (2 chunks, zero-padded to 256)
MY, MX = 14, 18            # one-hot moduli (exact, see header)
NCH = MY + MX              # 32 mask channels
D = DH + NCH               # 48 contraction channels
NEG = -1.0e9
GRP = 4                    # tiles per pipeline group
NG = NT // GRP             # 24 groups
LD = 12                    # tiles per input-DMA slice

BF16 = ml_dtypes.bfloat16

_Y_OK = {d % MY for d in range(-3, 4)}
_X_OK = {d % MX for d in range(-3, 4)}


@functools.lru_cache(maxsize=1)
def _mask_channels():
    """Constant aug channels.

    q side: [NCH, H, W] one-hots (y mod MY, x mod MX).
    k side: [NCH, PH, PW] veto values: 0 where the channel does not veto the
    pair, NEG where it does.
    """
    PH = PW = H + 2 * PAD
    qm = np.zeros((NCH, H, W), np.float32)
    yy = np.arange(H)
    xx = np.arange(W)
    for r in range(MY):
        qm[r, yy % MY == r, :] = 1.0
    for r in range(MX):
        qm[MY + r, :, xx % MX == r] = 1.0

    km = np.zeros((NCH, PH, PW), np.float32)
    py = np.arange(PH) - PAD   # absolute y of padded row
    px = np.arange(PW) - PAD
    for r in range(MY):
        bad = np.array([((y - r) % MY) not in _Y_OK for y in py])
        km[r, bad, :] = NEG
    for r in range(MX):
        bad = np.array([((x - r) % MX) not in _X_OK for x in px])
        km[MY + r, :, bad] = NEG
    return qm, km


def _host_prep(q, k, v):
    """Full [2,64,96,96] f32 inputs -> list of 8 per-core input dicts."""
    qm, km = _mask_channels()
    PH = PW = H + 2 * PAD
    in_maps = []
    for core in range(8):
        b, h = divmod(core, HEADS)
        qs = q[b, DH * h:DH * h + DH]          # [16, 96, 96]
        ks = k[b, DH * h:DH * h + DH]
        vs = v[b, DH * h:DH * h + DH]

        q_aug = np.empty((D, H, W), np.float32)
        q_aug[:DH] = qs
        q_aug[DH:] = qm
        # [D, GY, TY, GX, TX] -> [D, NT, Q]
        q_tiled = np.ascontiguousarray(
            q_aug.reshape(D, GY, TY, GX, TX).transpose(0, 1, 3, 2, 4)
            .reshape(D, NT, Q))

        k_aug = np.empty((D, PH, PW), np.float32)
        k_aug[:DH] = 0.0
        k_aug[:DH, PAD:PAD + H, PAD:PAD + W] = ks
        k_aug[DH:] = km
        # per-tile halo, flattened + padded to 256: [D, NT, 256]
        k_tiled = np.zeros((D, NT, 2 * KPC), np.float32)
        for ty in range(GY):
            for tx in range(GX):
                k_tiled[:, ty * GX + tx, :HALO] = k_aug[
                    :, TY * ty:TY * ty + HY,
                    TX * tx:TX * tx + HX].reshape(D, HALO)

        vt = np.zeros((PH, PW, DH + 1), np.float32)
        vt[PAD:PAD + H, PAD:PAD + W, :DH] = np.transpose(vs, (1, 2, 0))
        vt[PAD:PAD + H, PAD:PAD + W, DH] = 1.0
        # [128, NT, 2, 17], chunk c holds halo keys [128c : 128c+128)
        vt_res = np.zeros((KPC, NT, 2, DH + 1), np.float32)
        for ty in range(GY):
            for tx in range(GX):
                halo = vt[TY * ty:TY * ty + HY,
                          TX * tx:TX * tx + HX, :].reshape(HALO, DH + 1)
                vt_res[:, ty * GX + tx, 0] = halo[:KPC]
                vt_res[:HALO - KPC, ty * GX + tx, 1] = halo[KPC:]

        in_maps.append({
            "q_tiled": q_tiled.astype(BF16),
            "k_tiled": k_tiled.astype(BF16),
            "vt_res": vt_res.astype(BF16),
        })
    return in_maps


@functools.lru_cache(maxsize=1)
def _build_program():
    from contextlib import ExitStack
    import concourse.bass as bass
    import concourse.tile as tile
    from concourse import bacc, mybir

    f32 = mybir.dt.float32
    bf16 = mybir.dt.bfloat16

    nc = bacc.Bacc("TRN2", target_bir_lowering=False, debug=False,
                   num_devices=8)
    q_d = nc.dram_tensor("q_tiled", [D, NT, Q], bf16,
                         kind="ExternalInput").ap()
    k_d = nc.dram_tensor("k_tiled", [D, NT, 2 * KPC], bf16,
                         kind="ExternalInput").ap()
    vt_d = nc.dram_tensor("vt_res", [KPC, NT, 2, DH + 1], bf16,
                          kind="ExternalInput").ap()
    # flat [query-in-tile, tile, d] order; host un-permutes
    out_d = nc.dram_tensor("out", [Q, NT, DH], f32,
                           kind="ExternalOutput").ap()

    with tile.TileContext(nc) as tc:
        with ExitStack() as ctx:
            cpool = ctx.enter_context(tc.tile_pool(name="const", bufs=1))
            spool = ctx.enter_context(tc.tile_pool(name="sb", bufs=3))
            zpool = ctx.enter_context(tc.tile_pool(name="rz", bufs=3))
            pp_s = ctx.enter_context(
                tc.tile_pool(name="ps_scores", bufs=3, space="PSUM"))
            pp_o = ctx.enter_context(
                tc.tile_pool(name="ps_out", bufs=2, space="PSUM"))

            q_sb = cpool.tile([D, NT, Q], bf16)
            k_sb = cpool.tile([D, NT, 2 * KPC], bf16)
            vt_sb = cpool.tile([KPC, NT, 2, DH + 1], bf16)
            out_all = cpool.tile([Q, NT, DH], f32)
            # sliced loads so early groups unblock before the full load lands
            for g in range(NT // LD):
                s = slice(LD * g, LD * (g + 1))
                nc.sync.dma_start(q_sb[:, s, :], q_d[:, s, :])
                nc.sync.dma_start(k_sb[:, s, :], k_d[:, s, :])
                nc.sync.dma_start(vt_sb[:, s, :, :], vt_d[:, s, :, :])

            def c1(g):
                scores = pp_s.tile([128, 2 * GRP, 128], f32, tag="scores")
                for i in range(GRP):
                    t = GRP * g + i
                    for c in range(2):
                        nc.tensor.matmul(
                            scores[:, 2 * i + c, :Q],
                            lhsT=k_sb[:, t, KPC * c:KPC * (c + 1)],
                            rhs=q_sb[:, t, :],
                            start=True, stop=True)
                return scores

            # C1 runs two groups ahead of the exp/C2 consumers (3 PSUM bufs)
            # so the in-order PE queue always has ready C1 work before each
            # ACT-gated C2 group.
            sq = [c1(0), c1(1)]
            for g in range(NG):
                scores_cur = sq.pop(0)
                expT = spool.tile([128, 2 * GRP, 128], bf16, tag="expT")
                nc.scalar.activation(expT[:, :, :Q], scores_cur[:, :, :Q],
                                     mybir.ActivationFunctionType.Exp,
                                     scale=0.25)
                if g + 2 < NG:
                    sq.append(c1(g + 2))
                outu = pp_o.tile([128, GRP, 32], f32, tag="outu")
                for i in range(GRP):
                    t = GRP * g + i
                    for c in range(2):
                        nc.tensor.matmul(
                            outu[:, i, :DH + 1],
                            lhsT=expT[:, 2 * i + c, :],
                            rhs=vt_sb[:, t, c, :],
                            start=(c == 0), stop=(c == 1))
                rz = zpool.tile([Q, GRP, 1], f32, tag="rz")
                nc.vector.reciprocal(rz[:], outu[:Q, :, DH:DH + 1])
                u_b, rz_b = bass.broadcast_tensor_aps(
                    outu[:Q, :, :DH], rz[:, :, :])
                nc.vector.tensor_tensor(out_all[:, GRP * g:GRP * (g + 1), :],
                                        u_b, rz_b, mybir.AluOpType.mult)
            for g in range(NT // LD):
                s = slice(LD * g, LD * (g + 1))
                nc.gpsimd.dma_start(out_d[:, s, :], out_all[:, s, :])
    nc.compile()
    return nc


def kernel(q, k, v):
    from concourse.bass_utils import run_bass_kernel_spmd

    nc = _build_program()
    in_maps = _host_prep(np.asarray(q, np.float32), np.asarray(k, np.float32),
                         np.asarray(v, np.float32))
    res = run_bass_kernel_spmd(nc, in_maps, list(range(8)))

    out = np.empty((B, HEADS, DH, H, W), np.float32)
    for core in range(8):
        b, h = divmod(core, HEADS)
        # [Q, NT, DH] -> [qy,qx,ty,tx,d] -> [d,ty,qy,tx,qx] -> [DH,H,W]
        o = res.results[core]["out"].reshape(TY, TX, GY, GX, DH)
        out[b, h] = o.transpose(4, 2, 0, 3, 1).reshape(DH, H, W)
    return out.reshape(B, HEADS * DH, H, W)
